# revision 42
# baseline (speedup 1.0000x reference)
"""Multi-head attention (16 heads, B=4, L=1024, D=1024) on 8 TRN2 NeuronCores.

Sharding: core c = (batch b = c//2, head-half = c%2). Each core computes, for
its batch, the Q/K/V projections restricted to its 512 output columns
(8 heads), full attention for those heads over the batch's 1024 keys, and the
0.5*q + 0.5*ctx blend for its [1024, 512] output slice.

Device matmuls run in transposed layouts (contraction dim on partitions).
Q/K/scores use float32r operands (full PE rate at N>=256, ~1.5e-4 precision —
needed because softmax exponentiates score errors). V/expT/ctx use bf16
(attention weights and values tolerate 0.4% rounding).

Schedule (single phase, everything priority-interleaved):
- DMA stream in consumption order: (xq + m0 Q-weight cols), (xk + m0 K-weight
  cols) -> head-pair-0 scores and the exp pipeline (ACT is the ~74us
  bottleneck engine) start ~30us in; then remaining weight columns, then wv.
- The first-half residual trick: for half=1 cores the host permutes the
  contraction rows of xqT and Wq identically (matmuls invariant), so the
  residual rows 0.5*q always live in resident xq tiles 0..3 — no separate
  residual input.
- Softmax norm outputs are written into the dead qt/kt m-chunk regions
  (those are only read by the already-finished scores of the same m-chunk),
  so no late-phase SBUF pools exist and ctx/norm emission can interleave at
  full priority; expT is quad-granular (11 x 4KB slots) so the exp->ctx
  pipeline advances at sub-head depth.

Per-core layout:
  QT [d' 512, q 1024], KT [d' 512, kt 1024] (proj transposed, relu+bias)
  V_aug [kt 1024, 520] bf16; per head h: cols h*65..h*65+63 = V values,
     col h*65+64 = 2.0 (via the ones-row bias matmul), so the ctx matmul also
     produces a 2*sum(exp) row per head (flash-style).
  scoresT [kt, q] per head -> exp (no max-sub; scores in [0, 42]) -> expT
     (head pairs packed into PE row-groups 0-63/64-127, K=64 concurrency)
  ctxT_aug [65, 512] x2 per head; row 64 = 2*sumexp
  out = 0.5*q + ctx/(2*sumexp)
"""
import sys

sys.path.insert(0, "/opt/trn_rl_repo")

import numpy as np


def _build(nc_mod):
    bass, mybir, tile, bacc = nc_mod
    f32 = mybir.dt.float32
    f32r = mybir.dt.float32r
    bf16 = mybir.dt.bfloat16
    AF = mybir.ActivationFunctionType
    ALU = mybir.AluOpType

    D = 1024        # model dim / contraction dim
    DS = 512        # per-core output-column slice
    DSA = DS + 8    # with one aug column per head
    L = 1024        # sequence length (q and kt)
    KO = D // 128   # k chunks
    MQ = DS // 128  # m-chunks of d' slice (4)
    NQ = L // 512   # n-chunks of seq (2)
    NH = 8          # heads per core
    DH = 64
    VH = DSA // 2   # 260: V projection n-split, both halves fp32r-fast

    nc = bacc.Bacc("TRN2", target_bir_lowering=False, debug=False)
    with tile.TileContext(nc) as tc:
        with (
            tc.tile_pool(name="dram", bufs=1, space="DRAM") as dram,
            tc.tile_pool(name="persist", bufs=1) as sp,
            tc.tile_pool(name="expp", bufs=11) as ep,
            tc.tile_pool(name="bcp", bufs=1) as bcp,
            tc.tile_pool(name="pp1", bufs=4, space="PSUM") as pp1,
            tc.tile_pool(name="pp_sc", bufs=2, space="PSUM") as pp_sc,
            tc.tile_pool(name="xw", bufs=1) as xw,
        ):
            # ---- I/O ----
            xqT = dram.tile([D, L], f32r, kind="ExternalInput", name="xqT")
            xkT = dram.tile([D, L], f32r, kind="ExternalInput", name="xkT")
            wq = dram.tile([D, DS], f32r, kind="ExternalInput", name="wq")
            wk = dram.tile([D, DS], f32r, kind="ExternalInput", name="wk")
            wv = dram.tile([D, DSA], f32r, kind="ExternalInput", name="wv")
            bq = dram.tile([128, MQ], f32, kind="ExternalInput", name="bq")
            bk = dram.tile([128, MQ], f32, kind="ExternalInput", name="bk")
            bv = dram.tile([1, DSA], f32r, kind="ExternalInput", name="bv")
            ones = dram.tile([1, 128], f32r, kind="ExternalInput", name="ones")
            outT = dram.tile([DS, L], f32r, kind="ExternalOutput", name="outT")

            # ---- persistent SBUF ----
            qt_all = sp.tile([128, MQ, L], f32r)
            kt_all = sp.tile([128, MQ, L], f32r)
            v_all = sp.tile([128, KO, DSA], bf16)
            bcast = bcp.tile([DH, L], f32)

            bq_sb = xw.tile([128, MQ], f32)
            bk_sb = xw.tile([128, MQ], f32)
            bv_sb = xw.tile([1, DSA], f32r)
            ones_sb = xw.tile([1, 128], f32r)
            nc.sync.dma_start(bq_sb[:], bq[:])
            nc.sync.dma_start(bk_sb[:], bk[:])
            nc.sync.dma_start(bv_sb[:], bv[:])
            nc.sync.dma_start(ones_sb[:], ones[:])

            # preload the exp ACT table during the DMA phase
            dmy = xw.tile([1, 8], f32)
            nc.vector.memset(dmy[:], 0.0)
            dmy2 = xw.tile([1, 8], f32)
            nc.scalar.activation(dmy2[:], dmy[:], AF.Exp)

            xq_t, xk_t, wq_t, wk_t, wv_t = ([None] * KO for _ in range(5))
            # stream: (xq, wq-m0) then (xk, wk-m0) -> head-pair 0 unblocked
            # ~30us in; then m1-3 weight columns; wv last (V runs mid-flight)
            for k in range(KO):
                xq_t[k] = xw.tile([128, L], f32r, tag=f"xq{k}", name=f"xq_{k}")
                nc.sync.dma_start(xq_t[k][:], xqT[k * 128:(k + 1) * 128, :])
                wq_t[k] = xw.tile([128, DS], f32r, tag=f"wq{k}", name=f"wq_{k}")
                nc.sync.dma_start(wq_t[k][:, 0:128], wq[k * 128:(k + 1) * 128, 0:128])
            for k in range(KO):
                xk_t[k] = xw.tile([128, L], f32r, tag=f"xk{k}", name=f"xk_{k}")
                nc.sync.dma_start(xk_t[k][:], xkT[k * 128:(k + 1) * 128, :])
                wk_t[k] = xw.tile([128, DS], f32r, tag=f"wk{k}", name=f"wk_{k}")
                nc.sync.dma_start(wk_t[k][:, 0:128], wk[k * 128:(k + 1) * 128, 0:128])
            for k in range(KO):
                nc.sync.dma_start(wq_t[k][:, 128:DS], wq[k * 128:(k + 1) * 128, 128:DS])
                nc.sync.dma_start(wk_t[k][:, 128:DS], wk[k * 128:(k + 1) * 128, 128:DS])
            for k in range(KO):
                wv_t[k] = xw.tile([128, DSA], f32r, tag=f"wv{k}", name=f"wv_{k}")
                nc.sync.dma_start(wv_t[k][:], wv[k * 128:(k + 1) * 128, :])

            def proj_qk(m):
                for w_t, x_t, b_sb, dst in (
                    (wq_t, xq_t, bq_sb, qt_all),
                    (wk_t, xk_t, bk_sb, kt_all),
                ):
                    pss = [
                        pp1.tile([128, 512], f32, tag="p1",
                                 name=f"pj{m}{n}{dst.name[:2]}")
                        for n in range(NQ)
                    ]
                    for k in range(KO):
                        for n in range(NQ):
                            nc.tensor.matmul(
                                pss[n][:],
                                w_t[k][:, m * 128:(m + 1) * 128],
                                x_t[k][:, n * 512:(n + 1) * 512],
                                start=(k == 0), stop=(k == KO - 1),
                            )
                    for n in range(NQ):
                        # relu(x + bias) eviction -> fp32r
                        nc.vector.tensor_scalar(
                            dst[:, m, n * 512:(n + 1) * 512], pss[n][:],
                            b_sb[:, m:m + 1], 0.0, ALU.add, ALU.max,
                        )

            def emit_v_proj(ts):
                # V: out[kt 128, 520] = sum_k XkT[k,kt].T @ Wv_aug[k,:]
                #    + ones.T @ bv_aug, in two fp32r-fast N=260 halves
                for t in ts:
                    for c0 in (0, VH):
                        ps = pp1.tile([128, VH], f32, tag="p1", name=f"pv{t}_{c0}")
                        for k in range(KO):
                            nc.tensor.matmul(
                                ps[:], xk_t[k][:, t * 128:(t + 1) * 128],
                                wv_t[k][:, c0:c0 + VH], start=(k == 0), stop=False,
                            )
                        nc.tensor.matmul(ps[:], ones_sb[:], bv_sb[:, c0:c0 + VH],
                                         start=False, stop=True)
                        nc.vector.tensor_scalar(
                            v_all[:, t, c0:c0 + VH], ps[:], 0.0, None, ALU.max,
                        )

            # expT quad-granular ([128, 2 t-chunks, L] bf16 tiles): the
            # 11-slot pool pipelines exp/ctx at sub-head depth
            exp_q = [[None] * (KO // 2) for _ in range(NH)]

            def emit_scores_pair(j):
                # heads 2j (PE rows 0-63) and 2j+1 (rows 64-127), packed
                he, ho = 2 * j, 2 * j + 1
                for t in range(KO):
                    if t % 2 == 0:
                        exp_q[he][t // 2] = ep.tile(
                            [128, 2, L], bf16, tag="expT", name=f"eq{he}_{t // 2}")
                        exp_q[ho][t // 2] = ep.tile(
                            [128, 2, L], bf16, tag="expT", name=f"eq{ho}_{t // 2}")
                    pse = pp_sc.tile([128, L], f32, tag="sc", name=f"sc{he}_{t}")
                    pso = pp_sc.tile([128, L], f32, tag="sc", name=f"sc{ho}_{t}")
                    for n in range(NQ):
                        for ph, ps in ((0, pse), (DH, pso)):
                            nc.tensor.matmul(
                                ps[:, n * 512:(n + 1) * 512],
                                kt_all[ph:ph + DH, j, t * 128:(t + 1) * 128],
                                qt_all[ph:ph + DH, j, n * 512:(n + 1) * 512],
                                start=True, stop=True,
                            )
                    nc.scalar.activation(
                        exp_q[he][t // 2][:, t % 2, :], pse[:], AF.Exp)
                    nc.scalar.activation(
                        exp_q[ho][t // 2][:, t % 2, :], pso[:], AF.Exp)

            def emit_ctx(h):
                # ctxT_aug psums accumulate over kt; row 64 = 2*sum(exp).
                # Norm result lands in the dead qt m-chunk; recip scratch in
                # the dead kt m-chunk (row 127); combine + store at h odd.
                mh, ph = h // 2, (h % 2) * DH
                pss = [
                    pp1.tile([DH + 1, 512], f32, tag="p1", name=f"ctx{h}_{n}")
                    for n in range(NQ)
                ]
                for t in range(KO):
                    for n in range(NQ):
                        nc.tensor.matmul(
                            pss[n][:],
                            v_all[:, t, h * (DH + 1):(h + 1) * (DH + 1)],
                            exp_q[h][t // 2][:, t % 2, n * 512:(n + 1) * 512],
                            start=(t == 0), stop=(t == KO - 1),
                        )
                rrow = kt_all[0:1, mh, :]
                # f32r out is bit-identical f32; reduced rounding only at PE
                with nc.allow_low_precision(reason="f32r dest is f32-bit-exact"):
                    for n in range(NQ):
                        nc.vector.reciprocal(
                            rrow[:, n * 512:(n + 1) * 512], pss[n][DH:DH + 1, :])
                nc.gpsimd.partition_broadcast(bcast[:], rrow.bitcast(f32))
                for n in range(NQ):
                    nc.vector.tensor_tensor(
                        qt_all[ph:ph + DH, mh, n * 512:(n + 1) * 512],
                        pss[n][0:DH, :], bcast[:, n * 512:(n + 1) * 512],
                        ALU.mult,
                    )
                if h % 2 == 1:
                    # residual: xq tiles 0..3 hold the (host-permuted)
                    # contraction rows matching this core's output columns
                    nc.vector.tensor_scalar(
                        kt_all[:, mh, :], xq_t[mh][:], 0.5, None, ALU.mult,
                    )
                    nc.vector.tensor_tensor(
                        qt_all[:, mh, :], qt_all[:, mh, :], kt_all[:, mh, :],
                        ALU.add,
                    )
                    nc.sync.dma_start(
                        outT[mh * 128:(mh + 1) * 128, :], qt_all[:, mh, :])

            proj_qk(0)
            emit_scores_pair(0)
            proj_qk(1)
            emit_scores_pair(1)
            emit_v_proj(range(KO))
            emit_ctx(0)
            emit_ctx(1)
            proj_qk(2)
            emit_scores_pair(2)
            emit_ctx(2)
            emit_ctx(3)
            proj_qk(3)
            emit_scores_pair(3)
            for h in range(4, NH):
                emit_ctx(h)

    nc.compile()
    names = {
        "xqT": xqT.name, "xkT": xkT.name, "wq": wq.name, "wk": wk.name,
        "wv": wv.name, "bq": bq.name, "bk": bk.name, "bv": bv.name,
        "ones": ones.name, "outT": outT.name,
    }
    return nc, names


def _prep_in_maps(nm, queries, keys, Wq, bq, Wk, bk, Wv, bv):
    DS, DH, NH = 512, 64, 8
    in_maps = []
    for c in range(8):
        b, half = c // 2, c % 2
        sl = slice(half * DS, (half + 1) * DS)
        # interleaved augmented V weights/bias: per head 64 value cols + 1 aug
        wv_aug = np.zeros((1024, DS + NH), dtype=np.float32)
        bv_aug = np.zeros((1, DS + NH), dtype=np.float32)
        for h in range(NH):
            wv_aug[:, h * 65:h * 65 + DH] = Wv[:, half * DS + h * DH:half * DS + (h + 1) * DH]
            bv_aug[0, h * 65:h * 65 + DH] = bv[half * DS + h * DH:half * DS + (h + 1) * DH]
            bv_aug[0, h * 65 + DH] = 2.0
        # permute the contraction rows of xqT/Wq identically (matmul
        # invariant) so the residual rows land in xq tiles 0..3 on every core
        xqTc = np.ascontiguousarray(queries[b].T)
        wq_c = np.ascontiguousarray(Wq[:, sl])
        if half == 1:
            perm = np.r_[512:1024, 0:512]
            xqTc = np.ascontiguousarray(xqTc[perm])
            wq_c = np.ascontiguousarray(wq_c[perm])
        in_maps.append({
            nm["xqT"]: xqTc,
            nm["xkT"]: np.ascontiguousarray(keys[b].T),
            nm["wq"]: wq_c,
            nm["wk"]: np.ascontiguousarray(Wk[:, sl]),
            nm["wv"]: wv_aug,
            nm["bq"]: np.ascontiguousarray(bq[sl].reshape(4, 128).T),
            nm["bk"]: np.ascontiguousarray(bk[sl].reshape(4, 128).T),
            nm["bv"]: bv_aug,
            nm["ones"]: np.ones((1, 128), dtype=np.float32),
        })
    return in_maps


def kernel(queries, keys, Wq, bq, Wk, bk, Wv, bv):
    import concourse.bass as bass
    import concourse.mybir as mybir
    import concourse.tile as tile
    from concourse import bacc
    from concourse.bass_utils import run_bass_kernel_spmd

    queries = np.asarray(queries, dtype=np.float32)
    keys = np.asarray(keys, dtype=np.float32)
    Wq = np.asarray(Wq, dtype=np.float32)
    Wk = np.asarray(Wk, dtype=np.float32)
    Wv = np.asarray(Wv, dtype=np.float32)
    bq = np.asarray(bq, dtype=np.float32)
    bk = np.asarray(bk, dtype=np.float32)
    bv = np.asarray(bv, dtype=np.float32)

    B, L, D = queries.shape
    DS = 512

    nc, nm = _build((bass, mybir, tile, bacc))
    in_maps = _prep_in_maps(nm, queries, keys, Wq, bq, Wk, bk, Wv, bv)
    res = run_bass_kernel_spmd(nc, in_maps, core_ids=list(range(8)))

    out = np.empty((B, L, D), dtype=np.float32)
    for c in range(8):
        b, half = c // 2, c % 2
        out[b, :, half * DS:(half + 1) * DS] = res.results[c][nm["outT"]].T
    return out
